# revision 15
# baseline (speedup 1.0000x reference)
"""Multi-head attention Trainium2 kernel (B=4, S=2048, E=1024, H=16, D=64).

Sharding: head-parallel x data-parallel. Core c owns heads {2c, 2c+1} for all
4 batches -> 8 (batch, head) jobs per core, no cross-core communication.

Per (batch, head) job on device (all fp32):
  qT = (Wq_aug/8)^T @ xT_aug          [64, 2048]   (bias via ones-row in xT_aug)
  kT = Wk_aug^T @ xT_aug              [64, 2048]
  v  = xT_aug^T @ Wv_aug              [2048, 64]   (+ ones column -> [.., 65])
  scoresT[k, q] = kT_chunk^T @ qT     [128, 512] tiles  (= (q . k)/8 transposed)
  attnT = exp(scoresT)                wide ACT ops, [128, 4096] per 2 k-chunks
  outT[65, q] += v_aug_chunk^T @ attnT   accumulated over 16 k-chunks in PSUM;
                                          row 64 = sum_k attnT = softmax denom
  out = outT[0:64] * (1/outT[64])     reciprocal + K=1 ones-matmul broadcast
Host side only reshapes/transposes (sharding + unsharding), no math besides
the bias/scale folding into the weight matrices.
"""

import numpy as np

import concourse.bass as bass
import concourse.mybir as mybir
import concourse.tile as tile
from concourse.bass_utils import run_bass_kernel_spmd

F32 = mybir.dt.float32
F32R = mybir.dt.float32r

B, S, E, H = 4, 2048, 1024, 16
D = E // H            # 64
NCORES = 8
HPC = H // NCORES     # heads per core = 2
PAIRS = B * HPC       # jobs per core = 8
QG = 4                # q groups of 512
NQ = S // QG          # 512
KC = S // 128         # 16 k chunks of 128
GRP = 2               # k-chunks per exp group
NGRP = KC // GRP      # 8 exp groups per job


def _patched_drain_and_barrier(self, tick_clock, wait_clock):
    # This walrus build rejects >1 sync-wait on a Drain (CTRL) instruction.
    # Collect the TileContext-exit waits on individual NOPs instead.
    nc = self.nc
    collector = nc.sync.nop(nofuse=True)
    wait_clock.add_sem_waits(
        collector.ins, tile.ScopedClock({None: tick_clock.global_clock})
    )
    si = collector.ins.sync_info
    if si is not None and len(si.on_wait) > 1:
        waits = list(si.on_wait)
        collector.ins.sync_info = mybir.SyncInfo(
            on_wait=[waits[0]], on_update=list(si.on_update)
        )
        for w in waits[1:]:
            n2 = nc.sync.nop(nofuse=True)
            n2.ins.sync_info = mybir.SyncInfo(on_wait=[w], on_update=[])
    nc.sync.drain()
    nc.all_engine_barrier()
    popped = nc._tile_sem_poison_stack.pop()
    assert popped is self._sem_poison
    nc.clear_and_free_semaphores(list(self.sems.allocated().values()))
    nc.all_engine_barrier()


tile.TileContext._drain_and_barrier = _patched_drain_and_barrier

# Consecutive matmuls share stationary operands; the default
# --enable-ldw-opt=false forces a weight reload per matmul, serializing
# LDWEIGHTS with every MM. Enable the walrus LDW dedup.
from concourse import bass_utils as _bu  # noqa: E402

if not getattr(_bu, "_ldwopt_patched", False):
    _orig_run_command = _bu.run_command

    def _run_command_ldwopt(argv, **kw):
        import os
        if os.environ.get("BASS_LDWOPT", "1") == "1":
            argv = [
                "--enable-ldw-opt=true" if a == "--enable-ldw-opt=false" else a
                for a in argv
            ]
        return _orig_run_command(argv, **kw)

    _bu.run_command = _run_command_ldwopt
    _bu._ldwopt_patched = True

_MAX_WAITS = 1


def _split_excess_waits(nc):
    """This walrus build allows at most one sync-wait per instruction; hoist
    extra waits onto NOPs inserted immediately before, on the same engine."""
    n = 0
    for f in nc.m.functions:
        for bb in f.blocks:
            new_insts = []
            for inst in bb.instructions:
                si = inst.sync_info
                if si is not None and len(si.on_wait) > _MAX_WAITS:
                    waits = list(si.on_wait)
                    for w in waits[:-_MAX_WAITS]:
                        nop = mybir.InstNoOp(
                            name=f"waitnop-{n}",
                            engine=inst.engine,
                            ins=[],
                            outs=[],
                            sync_info=mybir.SyncInfo(on_wait=[w], on_update=[]),
                            bass_nofuse=True,
                        )
                        n += 1
                        new_insts.append(nop)
                    inst.sync_info = mybir.SyncInfo(
                        on_wait=waits[-_MAX_WAITS:],
                        on_update=list(si.on_update),
                    )
                new_insts.append(inst)
            bb.instructions = new_insts


_NC_CACHE = {}


def build_nc():
    if "nc" in _NC_CACHE:
        return _NC_CACHE["nc"]
    nc = bass.Bass()
    xt = nc.dram_tensor("xt", [PAIRS, D + 1, S], F32R, kind="ExternalInput")
    wq = nc.dram_tensor("wq", [HPC, D + 1, D], F32R, kind="ExternalInput")
    wk = nc.dram_tensor("wk", [HPC, D + 1, D], F32R, kind="ExternalInput")
    wv = nc.dram_tensor("wv", [HPC, D + 1, D], F32R, kind="ExternalInput")
    out = nc.dram_tensor("out", [PAIRS, D, S], F32, kind="ExternalOutput")

    QH = S // 2  # 1024: half the q range, so PSUM fits double-buffered scores

    with tile.TileContext(nc) as tc:
        with (
            tc.tile_pool(name="sb", bufs=2) as sb,
            tc.tile_pool(name="at", bufs=3) as atp,
            tc.tile_pool(name="wp", bufs=2) as wp,
            tc.tile_pool(name="cp", bufs=1) as cp,
            tc.tile_pool(name="sp", bufs=2, space="PSUM") as sp,
            tc.tile_pool(name="fp", bufs=2, space="PSUM") as fp,
            tc.tile_pool(name="op", bufs=1, space="PSUM") as op,
        ):
            # ones rows 0..64 so that ones[64:65, :] has base partition 64
            # (must match the rhs base partition in the broadcast matmul)
            ones = cp.tile([D + 1, D], F32R, tag="ones")
            nc.vector.memset(ones[:].bitcast(F32), 1.0)

            def load_pair(p):
                # inputs go on the sync queue; outputs use gpsimd so a
                # pending output DMA never blocks the next pair's prefetch
                jj = p % HPC
                xt_t = sb.tile([D + 1, S], F32R, tag="xt")
                nc.sync.dma_start(xt_t[:], xt[p])
                wq_t = wp.tile([D + 1, D], F32R, tag="wq")
                nc.sync.dma_start(wq_t[:], wq[jj])
                wk_t = wp.tile([D + 1, D], F32R, tag="wk")
                nc.sync.dma_start(wk_t[:], wk[jj])
                wv_t = wp.tile([D + 1, D], F32R, tag="wv")
                nc.sync.dma_start(wv_t[:], wv[jj])
                return xt_t, wq_t, wk_t, wv_t

            cur = load_pair(0)
            for p in range(PAIRS):
                xt_t, wq_t, wk_t, wv_t = cur

                # ---- projections (psum tiles share the "s" slots) ----
                qt = sb.tile([D, S], F32R, tag="qt")
                kt = sb.tile([D, S], F32R, tag="kt")
                for qg in range(QG):
                    sl = bass.ts(qg, NQ)
                    ps_q = sp.tile([128, 2 * NQ], F32, tag="s")
                    nc.tensor.matmul(ps_q[:D, :NQ], wq_t[:], xt_t[:, sl],
                                     start=True, stop=True)
                    nc.tensor.matmul(ps_q[:D, NQ:], wk_t[:], xt_t[:, sl],
                                     start=True, stop=True)
                    nc.vector.tensor_copy(qt[:, sl], ps_q[:D, :NQ])
                    nc.vector.tensor_copy(kt[:, sl], ps_q[:D, NQ:])

                # v with ones column: [128, 16*65]
                v_t = sb.tile([128, KC * (D + 1)], F32R, tag="v")
                nc.vector.memset(v_t[:].bitcast(F32), 1.0)
                for kc2 in range(KC // 2):
                    ps_v = sp.tile([128, 2 * NQ], F32, tag="s")
                    for h2 in range(2):
                        kc = 2 * kc2 + h2
                        nc.tensor.matmul(ps_v[:, h2 * NQ: h2 * NQ + D],
                                         xt_t[:, bass.ts(kc, 128)], wv_t[:],
                                         start=True, stop=True)
                        nc.vector.tensor_copy(
                            v_t[:, kc * (D + 1): kc * (D + 1) + D],
                            ps_v[:, h2 * NQ: h2 * NQ + D])

                # prefetch next pair's inputs while this pair computes
                if p + 1 < PAIRS:
                    cur = load_pair(p + 1)

                # ---- attention, one q-half at a time ----
                # software-pipelined: scores(kc+1) issue before out(kc) so
                # the PE never stalls on exp(kc)
                for qh in range(2):
                    q0 = qh * QH
                    out_ps = op.tile([D + 1, QH], F32, tag="out")
                    pend = None
                    for kc in range(KC):
                        ksl = bass.ts(kc, 128)
                        sps = sp.tile([128, 2 * NQ], F32, tag="s")
                        nc.tensor.matmul(sps[:, :NQ], kt[:, ksl],
                                         qt[:, q0: q0 + NQ],
                                         start=True, stop=True)
                        nc.tensor.matmul(sps[:, NQ:], kt[:, ksl],
                                         qt[:, q0 + NQ: q0 + 2 * NQ],
                                         start=True, stop=True)
                        # filler matmuls into a scratch PSUM bank keep the
                        # PE activity monitor warm (K=8/8) despite the
                        # ACT-gated cadence; results are never read
                        fl = fp.tile([D, NQ], F32, tag="fill")
                        nc.tensor.matmul(fl[:], ones[0:1, 0:D],
                                         qt[0:1, q0: q0 + NQ],
                                         start=True, stop=True)
                        at = atp.tile([128, 2 * NQ], F32R, tag="attn")
                        nc.scalar.activation(at[:], sps[:],
                                             mybir.ActivationFunctionType.Exp)
                        if pend is not None:
                            pat, pkc = pend
                            vsl = v_t[:, pkc * (D + 1): (pkc + 1) * (D + 1)]
                            nc.tensor.matmul(out_ps[:, :NQ], vsl, pat[:, :NQ],
                                             start=(pkc == 0), stop=False)
                            nc.tensor.matmul(out_ps[:, NQ:], vsl, pat[:, NQ:],
                                             start=(pkc == 0), stop=False)
                        pend = (at, kc)
                    pat, pkc = pend
                    vsl = v_t[:, pkc * (D + 1): (pkc + 1) * (D + 1)]
                    nc.tensor.matmul(out_ps[:, :NQ], vsl, pat[:, :NQ],
                                     start=False, stop=True)
                    nc.tensor.matmul(out_ps[:, NQ:], vsl, pat[:, NQ:],
                                     start=False, stop=True)

                    # ---- normalize: out[0:64] * (1 / out[64]) ----
                    o_t = sb.tile([D, QH], F32, tag="o")
                    for h2 in range(2):
                        sl = bass.ts(h2, NQ)
                        # denominators to SBUF (matmul rhs must be SBUF)
                        dn = sb.tile([D + 1, NQ], F32R, tag="dn")
                        nc.vector.tensor_copy(dn[D:D + 1, :],
                                              out_ps[D:D + 1, sl])
                        bc = sp.tile([128, 2 * NQ], F32, tag="s")
                        nc.tensor.matmul(bc[:D, :NQ], ones[D:D + 1, :],
                                         dn[D:D + 1, :], start=True, stop=True)
                        bc_sb = sb.tile([D, NQ], F32, tag="bc")
                        nc.vector.reciprocal(bc_sb[:], bc[:D, :NQ])
                        nc.vector.tensor_mul(o_t[:, sl], out_ps[:D, sl],
                                             bc_sb[:])
                    nc.gpsimd.dma_start(out[p, :, q0: q0 + QH], o_t[:])

    _split_excess_waits(nc)
    _NC_CACHE["nc"] = nc
    return nc


def _prep_inputs(sequences, Wq, bq, Wk, bk, Wv, bv):
    x = np.ascontiguousarray(np.asarray(sequences, dtype=np.float32))
    xh = x.reshape(B, S, H, D).transpose(2, 0, 3, 1)      # [H, B, D, S]
    aug = np.concatenate(
        [xh, np.ones((H, B, 1, S), np.float32)], axis=2)  # [H, B, 65, S]

    def augw(w, b_, scale=1.0):
        w = np.asarray(w, dtype=np.float32)
        b_ = np.asarray(b_, dtype=np.float32)
        return (np.concatenate([w, b_[:, None, :]], axis=1) * scale).astype(
            np.float32)

    wq_a = augw(Wq, bq, 1.0 / np.sqrt(D))                 # [H, 65, 64]
    wk_a = augw(Wk, bk)
    wv_a = augw(Wv, bv)

    in_maps = []
    for c in range(NCORES):
        xt_core = np.ascontiguousarray(np.stack(
            [aug[HPC * c + j, b] for b in range(B) for j in range(HPC)]))
        in_maps.append({
            "xt": xt_core,
            "wq": np.ascontiguousarray(wq_a[HPC * c: HPC * (c + 1)]),
            "wk": np.ascontiguousarray(wk_a[HPC * c: HPC * (c + 1)]),
            "wv": np.ascontiguousarray(wv_a[HPC * c: HPC * (c + 1)]),
        })
    return in_maps


def _assemble(results):
    out = np.empty((B, S, E), np.float32)
    for c in range(NCORES):
        r = results[c]["out"]                              # [8, 64, 2048]
        for b in range(B):
            for j in range(HPC):
                h = HPC * c + j
                out[b, :, h * D:(h + 1) * D] = r[HPC * b + j].T
    return out


def run(trace=False, **inputs):
    nc = build_nc()
    in_maps = _prep_inputs(**inputs)
    res = run_bass_kernel_spmd(nc, in_maps, list(range(NCORES)), trace=trace)
    return _assemble(res.results), res


def kernel(**inputs):
    out, _ = run(trace=False, **inputs)
    return out


# revision 17
# speedup vs baseline: 1.1482x; 1.1482x over previous
"""Multi-head attention Trainium2 kernel (B=4, S=2048, E=1024, H=16, D=64).

Sharding: head-parallel x data-parallel. Core c owns heads {2c, 2c+1} for all
4 batches -> 8 (batch, head) jobs per core, no cross-core communication.

Per (batch, head) job on device (all fp32):
  qT = (Wq_aug/8)^T @ xT_aug          [64, 2048]   (bias via ones-row in xT_aug)
  kT = Wk_aug^T @ xT_aug              [64, 2048]
  v  = xT_aug^T @ Wv_aug              [2048, 64]   (+ ones column -> [.., 65])
  scoresT[k, q] = kT_chunk^T @ qT     [128, 512] tiles  (= (q . k)/8 transposed)
  attnT = exp(scoresT)                wide ACT ops, [128, 4096] per 2 k-chunks
  outT[65, q] += v_aug_chunk^T @ attnT   accumulated over 16 k-chunks in PSUM;
                                          row 64 = sum_k attnT = softmax denom
  out = outT[0:64] * (1/outT[64])     reciprocal + K=1 ones-matmul broadcast
Host side only reshapes/transposes (sharding + unsharding), no math besides
the bias/scale folding into the weight matrices.
"""

import numpy as np

import concourse.bass as bass
import concourse.mybir as mybir
import concourse.tile as tile
from concourse.bass_utils import run_bass_kernel_spmd

F32 = mybir.dt.float32
F32R = mybir.dt.float32r

B, S, E, H = 4, 2048, 1024, 16
D = E // H            # 64
NCORES = 8
HPC = H // NCORES     # heads per core = 2
PAIRS = B * HPC       # jobs per core = 8
QG = 4                # q groups of 512
NQ = S // QG          # 512
KC = S // 128         # 16 k chunks of 128
GRP = 2               # k-chunks per exp group
NGRP = KC // GRP      # 8 exp groups per job


def _patched_drain_and_barrier(self, tick_clock, wait_clock):
    # This walrus build rejects >1 sync-wait on a Drain (CTRL) instruction.
    # Collect the TileContext-exit waits on individual NOPs instead.
    nc = self.nc
    collector = nc.sync.nop(nofuse=True)
    wait_clock.add_sem_waits(
        collector.ins, tile.ScopedClock({None: tick_clock.global_clock})
    )
    si = collector.ins.sync_info
    if si is not None and len(si.on_wait) > 1:
        waits = list(si.on_wait)
        collector.ins.sync_info = mybir.SyncInfo(
            on_wait=[waits[0]], on_update=list(si.on_update)
        )
        for w in waits[1:]:
            n2 = nc.sync.nop(nofuse=True)
            n2.ins.sync_info = mybir.SyncInfo(on_wait=[w], on_update=[])
    nc.sync.drain()
    nc.all_engine_barrier()
    popped = nc._tile_sem_poison_stack.pop()
    assert popped is self._sem_poison
    nc.clear_and_free_semaphores(list(self.sems.allocated().values()))
    nc.all_engine_barrier()


tile.TileContext._drain_and_barrier = _patched_drain_and_barrier

# Consecutive matmuls share stationary operands; the default
# --enable-ldw-opt=false forces a weight reload per matmul, serializing
# LDWEIGHTS with every MM. Enable the walrus LDW dedup.
from concourse import bass_utils as _bu  # noqa: E402

if not getattr(_bu, "_ldwopt_patched", False):
    _orig_run_command = _bu.run_command

    def _run_command_ldwopt(argv, **kw):
        import os
        if os.environ.get("BASS_LDWOPT", "1") == "1":
            argv = [
                "--enable-ldw-opt=true" if a == "--enable-ldw-opt=false" else a
                for a in argv
            ]
        return _orig_run_command(argv, **kw)

    _bu.run_command = _run_command_ldwopt
    _bu._ldwopt_patched = True

_MAX_WAITS = 1


def _split_excess_waits(nc):
    """This walrus build allows at most one sync-wait per instruction; hoist
    extra waits onto NOPs inserted immediately before, on the same engine."""
    n = 0
    for f in nc.m.functions:
        for bb in f.blocks:
            new_insts = []
            for inst in bb.instructions:
                si = inst.sync_info
                if si is not None and len(si.on_wait) > _MAX_WAITS:
                    waits = list(si.on_wait)
                    for w in waits[:-_MAX_WAITS]:
                        nop = mybir.InstNoOp(
                            name=f"waitnop-{n}",
                            engine=inst.engine,
                            ins=[],
                            outs=[],
                            sync_info=mybir.SyncInfo(on_wait=[w], on_update=[]),
                            bass_nofuse=True,
                        )
                        n += 1
                        new_insts.append(nop)
                    inst.sync_info = mybir.SyncInfo(
                        on_wait=waits[-_MAX_WAITS:],
                        on_update=list(si.on_update),
                    )
                new_insts.append(inst)
            bb.instructions = new_insts


_NC_CACHE = {}


def build_nc():
    if "nc" in _NC_CACHE:
        return _NC_CACHE["nc"]
    nc = bass.Bass()
    # x2[b] = [xA^T ; xB^T]: head A rows 0:64, head B rows 64:128
    x2 = nc.dram_tensor("x2", [B, 128, S], F32R, kind="ExternalInput")
    wq = nc.dram_tensor("wq", [HPC, D, D], F32R, kind="ExternalInput")
    wk = nc.dram_tensor("wk", [HPC, D, D], F32R, kind="ExternalInput")
    wv = nc.dram_tensor("wv", [HPC, D, D], F32R, kind="ExternalInput")
    bq = nc.dram_tensor("bq", [HPC, D, 1], F32, kind="ExternalInput")
    bk = nc.dram_tensor("bk", [HPC, D, 1], F32, kind="ExternalInput")
    bvb = nc.dram_tensor("bvb", [HPC, 128, D], F32, kind="ExternalInput")
    out = nc.dram_tensor("out", [PAIRS, D, S], F32, kind="ExternalOutput")

    QH = S // 2  # 1024: half the q range, so PSUM fits everything

    with tile.TileContext(nc) as tc:
        with (
            tc.tile_pool(name="sb", bufs=2) as sb,
            tc.tile_pool(name="at", bufs=4) as atp,
            tc.tile_pool(name="wp", bufs=1) as wp,
            tc.tile_pool(name="cp", bufs=1) as cp,
            tc.tile_pool(name="spa", bufs=1, space="PSUM") as spa,
            tc.tile_pool(name="spb", bufs=1, space="PSUM") as spb,
            tc.tile_pool(name="opa", bufs=1, space="PSUM") as opa,
            tc.tile_pool(name="opb", bufs=1, space="PSUM") as opb,
        ):
            # ones rows 0..64 so that ones[64:65, :] has base partition 64
            # (must match the rhs base partition in the broadcast matmul)
            ones = cp.tile([D + 1, D], F32R, tag="ones")
            nc.vector.memset(ones[:].bitcast(F32), 1.0)

            # weights resident for the whole kernel; head B weight tiles live
            # on partitions 64:128 to match its contraction rows
            w_t = {}
            for nm, dram in (("wq", wq), ("wk", wk), ("wv", wv)):
                t = wp.tile([128, D], F32R, tag=nm)
                nc.sync.dma_start(t[:D, :], dram[0])
                nc.sync.dma_start(t[D:128, :], dram[1])
                w_t[nm] = t
            b_t = {}
            for nm, dram in (("bq", bq), ("bk", bk)):
                t = wp.tile([D, 1], F32, tag=nm + "0")
                nc.sync.dma_start(t[:], dram[0])
                b_t[nm, 0] = t
                t = wp.tile([D, 1], F32, tag=nm + "1")
                nc.sync.dma_start(t[:], dram[1])
                b_t[nm, 1] = t
            bvb_t = {}
            for jj in range(HPC):
                t = wp.tile([128, D], F32, tag=f"bvb{jj}")
                nc.sync.dma_start(t[:], bvb[jj])
                bvb_t[jj] = t

            def load_b(b):
                t = sb.tile([128, S], F32R, tag="x2")
                nc.sync.dma_start(t[:], x2[b])
                return t

            cur = load_b(0)
            for b in range(B):
                x2_t = cur

                # ---- q/k projections, row-packed across the two heads ----
                # head A contracts partitions 0:64, head B partitions 64:128;
                # the two matmuls run concurrently in the PE array halves.
                # Biases are added while draining PSUM; head B's result is
                # then DMA-shifted onto partitions 64:128.
                qt = sb.tile([128, S], F32R, tag="qt")
                kt = sb.tile([128, S], F32R, tag="kt")
                for qg in range(QG):
                    sl = bass.ts(qg, NQ)
                    ps = spa.tile([128, 2 * NQ], F32, tag="sa")
                    ps2 = spb.tile([128, 2 * NQ], F32, tag="sb")
                    nc.tensor.matmul(ps[:D, :NQ], w_t["wq"][:D, :],
                                     x2_t[:D, sl], start=True, stop=True)
                    nc.tensor.matmul(ps2[:D, :NQ], w_t["wq"][D:128, :],
                                     x2_t[D:128, sl], start=True, stop=True)
                    nc.tensor.matmul(ps[:D, NQ:], w_t["wk"][:D, :],
                                     x2_t[:D, sl], start=True, stop=True)
                    nc.tensor.matmul(ps2[:D, NQ:], w_t["wk"][D:128, :],
                                     x2_t[D:128, sl], start=True, stop=True)
                    nc.vector.tensor_scalar_add(qt[:D, sl], ps[:D, :NQ],
                                                b_t["bq", 0][:])
                    nc.vector.tensor_scalar_add(kt[:D, sl], ps[:D, NQ:],
                                                b_t["bk", 0][:])
                    tmq = sb.tile([D, NQ], F32R, tag="tmq")
                    tmk = sb.tile([D, NQ], F32R, tag="tmk")
                    nc.vector.tensor_scalar_add(tmq[:], ps2[:D, :NQ],
                                                b_t["bq", 1][:])
                    nc.vector.tensor_scalar_add(tmk[:], ps2[:D, NQ:],
                                                b_t["bk", 1][:])
                    nc.sync.dma_start(qt[D:128, sl], tmq[:])
                    nc.sync.dma_start(kt[D:128, sl], tmk[:])

                # ---- v projections, row-packed, with ones column ----
                vA = sb.tile([128, KC * (D + 1)], F32R, tag="v0")
                vB = sb.tile([128, KC * (D + 1)], F32R, tag="v1")
                nc.vector.memset(vA[:].bitcast(F32), 1.0)
                nc.vector.memset(vB[:].bitcast(F32), 1.0)
                for kc2 in range(KC // 2):
                    ps_v = spa.tile([128, 2 * NQ], F32, tag="sa")
                    ps_v2 = spb.tile([128, 2 * NQ], F32, tag="sb")
                    for h2 in range(2):
                        kc = 2 * kc2 + h2
                        ksl = bass.ts(kc, 128)
                        nc.tensor.matmul(ps_v[:, h2 * NQ: h2 * NQ + D],
                                         x2_t[:D, ksl], w_t["wv"][:D, :],
                                         start=True, stop=True)
                        nc.tensor.matmul(ps_v2[:, h2 * NQ: h2 * NQ + D],
                                         x2_t[D:128, ksl],
                                         w_t["wv"][D:128, :],
                                         start=True, stop=True)
                        nc.vector.tensor_add(
                            vA[:, kc * (D + 1): kc * (D + 1) + D],
                            ps_v[:, h2 * NQ: h2 * NQ + D], bvb_t[0][:])
                        nc.vector.tensor_add(
                            vB[:, kc * (D + 1): kc * (D + 1) + D],
                            ps_v2[:, h2 * NQ: h2 * NQ + D], bvb_t[1][:])

                # prefetch next batch's inputs while this one computes
                if b + 1 < B:
                    cur = load_b(b + 1)

                # ---- attention, one q-half at a time, both heads ----
                # scores row-packed (A in PE rows 0:64, B in 64:128,
                # concurrent); exp per head; out accumulation per head.
                # Out matmuls for chunk kc are emitted after scores of kc+1
                # so the PE never waits on the exp.
                for qh in range(2):
                    q0 = qh * QH
                    outA = opa.tile([D + 1, QH], F32, tag="oa")
                    outB = opb.tile([D + 1, QH], F32, tag="ob")
                    pend = None
                    for kc in range(KC):
                        ksl = bass.ts(kc, 128)
                        sA = spa.tile([128, 2 * NQ], F32, tag="sa")
                        sB = spb.tile([128, 2 * NQ], F32, tag="sb")
                        for h2 in range(2):
                            qsl = slice(q0 + h2 * NQ, q0 + (h2 + 1) * NQ)
                            psl = bass.ts(h2, NQ)
                            nc.tensor.matmul(sA[:, psl], kt[:D, ksl],
                                             qt[:D, qsl],
                                             start=True, stop=True)
                            nc.tensor.matmul(sB[:, psl], kt[D:128, ksl],
                                             qt[D:128, qsl],
                                             start=True, stop=True)
                        aA = atp.tile([128, 2 * NQ], F32R, tag="attn")
                        nc.scalar.activation(aA[:], sA[:],
                                             mybir.ActivationFunctionType.Exp)
                        aB = atp.tile([128, 2 * NQ], F32R, tag="attn")
                        nc.scalar.activation(aB[:], sB[:],
                                             mybir.ActivationFunctionType.Exp)
                        if pend is not None:
                            pA, pB, pkc = pend
                            for ops_, vt, pa in ((outA, vA, pA),
                                                 (outB, vB, pB)):
                                vsl = vt[:, pkc * (D + 1): (pkc + 1) * (D + 1)]
                                nc.tensor.matmul(ops_[:, :NQ], vsl, pa[:, :NQ],
                                                 start=(pkc == 0), stop=False)
                                nc.tensor.matmul(ops_[:, NQ:], vsl, pa[:, NQ:],
                                                 start=(pkc == 0), stop=False)
                        pend = (aA, aB, kc)
                    pA, pB, pkc = pend
                    for ops_, vt, pa in ((outA, vA, pA), (outB, vB, pB)):
                        vsl = vt[:, pkc * (D + 1): (pkc + 1) * (D + 1)]
                        nc.tensor.matmul(ops_[:, :NQ], vsl, pa[:, :NQ],
                                         start=False, stop=True)
                        nc.tensor.matmul(ops_[:, NQ:], vsl, pa[:, NQ:],
                                         start=False, stop=True)

                    # ---- normalize: out[0:64] * (1 / out[64]) ----
                    for jj, ops_ in ((0, outA), (1, outB)):
                        o_t = sb.tile([D, QH], F32, tag=f"o{jj}")
                        pool = spb if jj else spa
                        for h2 in range(2):
                            sl = bass.ts(h2, NQ)
                            dn = sb.tile([D + 1, NQ], F32R, tag="dn")
                            nc.vector.tensor_copy(dn[D:D + 1, :],
                                                  ops_[D:D + 1, sl])
                            bc = pool.tile([128, 2 * NQ], F32,
                                           tag="sb" if jj else "sa")
                            nc.tensor.matmul(bc[:D, :NQ], ones[D:D + 1, :],
                                             dn[D:D + 1, :],
                                             start=True, stop=True)
                            bc_sb = sb.tile([D, NQ], F32, tag="bc")
                            nc.vector.reciprocal(bc_sb[:], bc[:D, :NQ])
                            nc.vector.tensor_mul(o_t[:, sl], ops_[:D, sl],
                                                 bc_sb[:])
                        nc.gpsimd.dma_start(
                            out[HPC * b + jj, :, q0: q0 + QH], o_t[:])

    _split_excess_waits(nc)
    _NC_CACHE["nc"] = nc
    return nc


def _prep_inputs(sequences, Wq, bq, Wk, bk, Wv, bv):
    x = np.ascontiguousarray(np.asarray(sequences, dtype=np.float32))
    xh = x.reshape(B, S, H, D).transpose(2, 0, 3, 1)      # [H, B, D, S]

    scale = np.float32(1.0 / np.sqrt(D))
    Wq = np.asarray(Wq, np.float32) * scale
    bq = np.asarray(bq, np.float32) * scale
    Wk = np.asarray(Wk, np.float32)
    bk = np.asarray(bk, np.float32)
    Wv = np.asarray(Wv, np.float32)
    bv = np.asarray(bv, np.float32)

    in_maps = []
    for c in range(NCORES):
        hA, hB = HPC * c, HPC * c + 1
        x2 = np.ascontiguousarray(
            np.concatenate([xh[hA], xh[hB]], axis=1))     # [B, 128, S]
        bvb = np.ascontiguousarray(np.broadcast_to(
            np.stack([bv[hA], bv[hB]])[:, None, :], (HPC, 128, D)))
        in_maps.append({
            "x2": x2,
            "wq": np.ascontiguousarray(Wq[hA:hB + 1]),
            "wk": np.ascontiguousarray(Wk[hA:hB + 1]),
            "wv": np.ascontiguousarray(Wv[hA:hB + 1]),
            "bq": np.ascontiguousarray(bq[hA:hB + 1, :, None]),
            "bk": np.ascontiguousarray(bk[hA:hB + 1, :, None]),
            "bvb": bvb,
        })
    return in_maps


def _assemble(results):
    out = np.empty((B, S, E), np.float32)
    for c in range(NCORES):
        r = results[c]["out"]                              # [8, 64, 2048]
        for b in range(B):
            for j in range(HPC):
                h = HPC * c + j
                out[b, :, h * D:(h + 1) * D] = r[HPC * b + j].T
    return out


def run(trace=False, **inputs):
    nc = build_nc()
    in_maps = _prep_inputs(**inputs)
    res = run_bass_kernel_spmd(nc, in_maps, list(range(NCORES)), trace=trace)
    return _assemble(res.results), res


def kernel(**inputs):
    out, _ = run(trace=False, **inputs)
    return out
